# revision 3
# baseline (speedup 1.0000x reference)
"""Trainium2 Bass kernel for nn_CortexBlock_59940563583556.

Math note (exact, not an approximation): the reference initializes the
fast-weight state U0 = V0 = 0 inside reference() itself, and every term
of the scan's update to U/V is proportional to ku = k_t^T @ U (zero when
U == 0).  By induction U_t == V_t == 0 for the whole scan, for ANY input
values.  Hence k_fast == 0, score_fast == 0, and (since mix_logit is
added to both logits, softmax is shift-invariant) the block reduces
exactly to:

    q = h @ Wq.T ; k = h @ Wk.T ; v = h @ Wv.T          (per-head split)
    g[b,t,h]  = sigmoid( sum_d q[b,t,h,d] * k[b,t,h,d] / sqrt(64) )
    out       = (g * v  per head) @ Wo.T

m_gate / alpha_scale / Wa / ba / mix_logit do not affect the output.

Sharding: data-parallel over the 8192 rows of the flattened [B*T, D]
activations across 8 NeuronCores (1024 rows each); the four 1024x1024
weight matrices are replicated.

All operand prep happens on the HOST: weights and activations are cast
to bf16 and pre-transposed into the exact SBUF layouts the PE needs
([kt, 128, n] with the contraction dim on partitions), so the device
does zero staging work -- no f32 loads, no casts, no weight transposes.
This halves HBM traffic and removes ~35us of PE transpose/copy work vs
the previous version.

Per-core dataflow:
  - resident SBUF: wq/wk/wv/wo as W^T [128, kt, 1024] bf16, hT
    [128, kt, 1024] bf16 (DMA'd in kt-slices for fine-grained deps).
  - pass 1, per 128-row tile, kt-OUTER loop: one stationary hT block
    feeds 6 matmuls (q/k/v x two 512-col chunks) before the next
    LDWEIGHTS, accumulating 6 PSUM banks over the 8 kt steps.
    Then: g = sigmoid(per-head rowsum(q*k)/8) (ACT+DVE), y = g*v (DVE,
    bf16), yT via DMA transpose.
  - pass 2, per tile: out = y @ Wo.T with yT stationary (kt-outer, 2
    chunks), PSUM->SBUF copy on ACT, DMA out (f32).
"""

import numpy as np
import ml_dtypes

import concourse.bass as bass
import concourse.mybir as mybir
import concourse.tile as tile
from concourse import bacc
from concourse.bass_utils import run_bass_kernel_spmd

F32 = mybir.dt.float32
BF16 = mybir.dt.bfloat16

N_CORES = 8
D = 1024          # model dim
ROWS = 8192       # B*T
M_CORE = ROWS // N_CORES   # rows per core
P = 128           # partitions
KT = D // P       # contraction tiles
MT = M_CORE // P  # row tiles per core
NCH = 2           # output-column chunks of 512
CHW = D // NCH    # 512
H = 16            # heads
DH = 64           # head dim
INV_SQRT_DH = 1.0 / (DH ** 0.5)

_COMPILED = None  # (nc,) cache
LAST_RESULT = None  # BassKernelResults of the most recent run (for test harness)


def _build():
    nc = bacc.Bacc("TRN2", target_bir_lowering=False, debug=False)

    hT_in = nc.dram_tensor("ht", [KT, P, M_CORE], BF16, kind="ExternalInput")
    w_in = {
        name: nc.dram_tensor(name, [KT, P, D], BF16, kind="ExternalInput")
        for name in ("wq", "wk", "wv", "wo")
    }
    out = nc.dram_tensor("out", [M_CORE, D], F32, kind="ExternalOutput")

    with tile.TileContext(nc) as tc:
        with (
            tc.tile_pool(name="res", bufs=1) as res_pool,
            tc.tile_pool(name="qsb", bufs=2) as q_pool,
            tc.tile_pool(name="sp", bufs=2) as sp_pool,
            tc.tile_pool(name="small", bufs=4) as small_pool,
            tc.tile_pool(name="y", bufs=2) as y_pool,
            tc.tile_pool(name="yT", bufs=MT) as yT_pool,
            tc.tile_pool(name="osb", bufs=2) as o_pool,
            tc.tile_pool(name="qkv_ps", bufs=1, space="PSUM") as qkv_ps,
            tc.tile_pool(name="o_ps", bufs=1, space="PSUM") as o_ps,
        ):
            # ---- resident operands, host-prepped layouts ----
            wsb = {
                name: res_pool.tile([P, KT, D], BF16, tag=f"w_{name}",
                                    name=f"w_{name}")
                for name in ("wq", "wk", "wv", "wo")
            }
            hsb = res_pool.tile([P, KT, M_CORE], BF16, tag="h", name="h")

            # kt-sliced loads so the first row-tile's matmuls can start as
            # slices land; qkv weights + h split across three queues.
            for kt in range(KT):
                nc.scalar.dma_start(out=wsb["wq"][:, kt, :], in_=w_in["wq"][kt])
                nc.sync.dma_start(out=wsb["wk"][:, kt, :], in_=w_in["wk"][kt])
                nc.scalar.dma_start(out=wsb["wv"][:, kt, :], in_=w_in["wv"][kt])
                nc.gpsimd.dma_start(out=hsb[:, kt, :], in_=hT_in[kt])
            for kt in range(KT):
                nc.sync.dma_start(out=wsb["wo"][:, kt, :], in_=w_in["wo"][kt])

            # ---- pass 1: q/k/v projections + gating + yT, per row tile ----
            yT_tiles = []
            for i in range(MT):
                ms = slice(i * P, (i + 1) * P)
                qp = [qkv_ps.tile([P, CHW], F32, tag=f"q{jo}", name=f"q{jo}")
                      for jo in range(NCH)]
                kp = [qkv_ps.tile([P, CHW], F32, tag=f"k{jo}", name=f"k{jo}")
                      for jo in range(NCH)]
                vp = [qkv_ps.tile([P, CHW], F32, tag=f"v{jo}", name=f"v{jo}")
                      for jo in range(NCH)]
                for kt in range(KT):
                    lhs = hsb[:, kt, ms]
                    for jo in range(NCH):
                        js = slice(jo * CHW, (jo + 1) * CHW)
                        for ps_t, wname in ((qp[jo], "wq"), (kp[jo], "wk"),
                                            (vp[jo], "wv")):
                            nc.tensor.matmul(
                                out=ps_t,
                                lhsT=lhs,
                                rhs=wsb[wname][:, kt, js],
                                start=(kt == 0),
                                stop=(kt == KT - 1),
                            )

                # s[m, h] = sum_{d in head} q*k ; g = sigmoid(s/8)
                # (DVE reads one PSUM operand max: stage q in SBUF as bf16)
                sp = sp_pool.tile([P, D], F32, tag="sp")
                for jo in range(NCH):
                    js = slice(jo * CHW, (jo + 1) * CHW)
                    qsb = q_pool.tile([P, CHW], BF16, tag="qsb")
                    nc.scalar.copy(out=qsb, in_=qp[jo])
                    nc.vector.tensor_mul(out=sp[:, js], in0=qsb, in1=kp[jo])
                s = small_pool.tile([P, H], F32, tag="s")
                nc.vector.reduce_sum(
                    out=s,
                    in_=sp.rearrange("p (h d) -> p h d", d=DH),
                    axis=mybir.AxisListType.X,
                )
                g = small_pool.tile([P, H], F32, tag="g")
                nc.scalar.activation(
                    out=g, in_=s,
                    func=mybir.ActivationFunctionType.Sigmoid,
                    scale=INV_SQRT_DH,
                )

                # y = g (broadcast over head dim) * v, in bf16
                y = y_pool.tile([P, D], BF16, tag="y")
                for jo in range(NCH):
                    g_sl = g[:, jo * (H // NCH):(jo + 1) * (H // NCH)]
                    g_bc = bass.AP(
                        tensor=g_sl.tensor, offset=g_sl.offset,
                        ap=[*g_sl.ap, [0, DH]],
                    )
                    nc.vector.tensor_mul(
                        out=y[:, jo * CHW:(jo + 1) * CHW].rearrange(
                            "p (h d) -> p h d", d=DH),
                        in0=vp[jo].rearrange("p (h d) -> p h d", d=DH),
                        in1=g_bc,
                    )

                yT = yT_pool.tile([P, KT, P], BF16, tag="yT")
                nc.sync.dma_start_transpose(out=yT, in_=y)
                yT_tiles.append(yT)

            # ---- pass 2: out = y @ Wo.T per tile, yT stationary ----
            for i in range(MT):
                ms = slice(i * P, (i + 1) * P)
                op = [o_ps.tile([P, CHW], F32, tag=f"o{jo}", name=f"o{jo}")
                      for jo in range(NCH)]
                for kt in range(KT):
                    for jo in range(NCH):
                        js = slice(jo * CHW, (jo + 1) * CHW)
                        nc.tensor.matmul(
                            out=op[jo],
                            lhsT=yT_tiles[i][:, kt, :],
                            rhs=wsb["wo"][:, kt, js],
                            start=(kt == 0),
                            stop=(kt == KT - 1),
                        )
                osb = o_pool.tile([P, D], F32, tag="osb")
                for jo in range(NCH):
                    js = slice(jo * CHW, (jo + 1) * CHW)
                    nc.scalar.copy(out=osb[:, js], in_=op[jo])
                nc.gpsimd.dma_start(out=out[ms, :], in_=osb)

    nc.compile()
    return nc


def kernel(hidden_states, m_gate, alpha_scale, Wq, Wk, Wv, Wo, Wa, ba, mix_logit,
           **_unused):
    global _COMPILED, LAST_RESULT
    if _COMPILED is None:
        _COMPILED = _build()
    nc = _COMPILED

    bf16 = ml_dtypes.bfloat16
    h = np.asarray(hidden_states, dtype=np.float32).reshape(ROWS, D)

    def prep_w(w):
        # W [j, d] -> W^T [kt, p, j] bf16: wT[kt, p, j] = W[j, kt*128+p]
        return np.ascontiguousarray(
            np.asarray(w, dtype=np.float32).T.reshape(KT, P, D)).astype(bf16)

    wq, wk, wv, wo = (prep_w(w) for w in (Wq, Wk, Wv, Wo))

    in_maps = []
    for c in range(N_CORES):
        hc = h[c * M_CORE:(c + 1) * M_CORE]  # [M_CORE, D]
        # hT [kt, p, m] = h[m, kt*128+p]
        ht = np.ascontiguousarray(hc.T.reshape(KT, P, M_CORE)).astype(bf16)
        in_maps.append({"ht": ht, "wq": wq, "wk": wk, "wv": wv, "wo": wo})

    res = run_bass_kernel_spmd(nc, in_maps, core_ids=list(range(N_CORES)))
    LAST_RESULT = res
    out = np.concatenate([res.results[c]["out"] for c in range(N_CORES)], axis=0)
    B, T = 4, 2048
    return out.reshape(B, T, D)


# revision 6
# speedup vs baseline: 1.1560x; 1.1560x over previous
"""Trainium2 Bass kernel for nn_CortexBlock_59940563583556.

Math note (exact, not an approximation): the reference initializes the
fast-weight state U0 = V0 = 0 inside reference() itself, and every term
of the scan's update to U/V is proportional to ku = k_t^T @ U (zero when
U == 0).  By induction U_t == V_t == 0 for the whole scan, for ANY input
values.  Hence k_fast == 0, score_fast == 0, and (since mix_logit is
added to both logits, softmax is shift-invariant) the block reduces
exactly to:

    q = h @ Wq.T ; k = h @ Wk.T ; v = h @ Wv.T          (per-head split)
    g[b,t,h]  = sigmoid( sum_d q[b,t,h,d] * k[b,t,h,d] / sqrt(64) )
    out       = (g * v  per head) @ Wo.T

m_gate / alpha_scale / Wa / ba / mix_logit do not affect the output.

Sharding: data-parallel over the 8192 rows of the flattened [B*T, D]
activations across 8 NeuronCores (1024 rows each); weights replicated.

All operand prep happens on the HOST: weights and activations are cast
to bf16 and pre-transposed into the exact SBUF layouts the PE needs
(contraction dim on partitions), so the device does zero staging work.

Early-kernel DMA is the scarce resource (~250 GB/s aggregate while the
rings ramp), so the PE schedule is ordered so each arriving 256KB
weight slice enables >= 1.7us of matmuls:
  - phase A: q-projections for row tiles 0-3, kt-outer ACROSS tiles
    (8 matmuls per wq slice; wq slices alternate between both HWDGE
    rings).
  - phase B: fused k+v for tiles 0-1, kt-outer across tiles (wk on
    sync ring, wv on scalar ring stream in parallel).
  - phase C: k for tiles 2-3, then v for tiles 2-3 (weights resident
    by now); gating chains for tiles 0-3 run during C/D.
  - phase D: tiles 4-7, full q/k/v per tile (6 matmuls per stationary
    hT block).
  - pass 2: out = y @ Wo.T per tile (yT from DMA transpose); PSUM
    copies split ACT/DVE; out DMA'd per 512-col chunk on scalar ring.
PSUM: 8 banks as 4 pairs, manually scheduled (bank-reuse chains in
comments) so write-after-read waits stay off the PE critical path.
Engine balance: qsb/osb-jo0 copies + sigmoid on ACT, qsb-jo1 on ACT,
sp-muls/reduce/y-muls + osb-jo1 on DVE; sp is bf16 (2x reduce rate).
"""

import numpy as np
import ml_dtypes

import concourse.bass as bass
import concourse.mybir as mybir
import concourse.tile as tile
from concourse import bacc
from concourse.bass_utils import run_bass_kernel_spmd

F32 = mybir.dt.float32
BF16 = mybir.dt.bfloat16

N_CORES = 8
D = 1024          # model dim
ROWS = 8192       # B*T
M_CORE = ROWS // N_CORES   # rows per core
P = 128           # partitions
KT = D // P       # contraction tiles
MT = M_CORE // P  # row tiles per core
NCH = 2           # output-column chunks of 512
CHW = D // NCH    # 512
H = 16            # heads
DH = 64           # head dim
INV_SQRT_DH = 1.0 / (DH ** 0.5)

_COMPILED = None
LAST_RESULT = None  # BassKernelResults of the most recent run (for test harness)


def _build():
    nc = bacc.Bacc("TRN2", target_bir_lowering=False, debug=False)

    hT_in = nc.dram_tensor("ht", [KT, P, M_CORE], BF16, kind="ExternalInput")
    w_in = {
        name: nc.dram_tensor(name, [KT, P, D], BF16, kind="ExternalInput")
        for name in ("wq", "wk", "wv", "wo")
    }
    out = nc.dram_tensor("out", [M_CORE, D], F32, kind="ExternalOutput")

    with tile.TileContext(nc) as tc:
        with (
            tc.tile_pool(name="res", bufs=1) as res_pool,
            tc.tile_pool(name="qsb", bufs=4) as q_pool,
            tc.tile_pool(name="sp", bufs=2) as sp_pool,
            tc.tile_pool(name="small", bufs=4) as small_pool,
            tc.tile_pool(name="y", bufs=2) as y_pool,
            tc.tile_pool(name="yT", bufs=MT) as yT_pool,
            tc.tile_pool(name="osb", bufs=2) as o_pool,
            tc.tile_pool(name="ps", bufs=1, space="PSUM") as ps_pool,
        ):
            # ---- resident operands, host-prepped layouts ----
            wsb = {
                name: res_pool.tile([P, KT, D], BF16, tag=f"w_{name}",
                                    name=f"w_{name}")
                for name in ("wq", "wk", "wv", "wo")
            }
            hsb = res_pool.tile([P, KT, M_CORE], BF16, tag="h", name="h")

            # DMA schedule, ordered by first-need time.
            #   scalar ring: wq evens | wv 0..7 | wo[0:4] | out chunks
            #   sync ring:   wq odds  | wk 0..7 | wo[4:8] | yT transposes
            #   gpsimd:      hT cols 0:512 (tiles 0-3), then cols 512:1024
            for kt in range(0, KT, 2):
                nc.scalar.dma_start(out=wsb["wq"][:, kt, :], in_=w_in["wq"][kt])
                nc.sync.dma_start(out=wsb["wq"][:, kt + 1, :],
                                  in_=w_in["wq"][kt + 1])
            for kt in range(KT):
                nc.scalar.dma_start(out=wsb["wv"][:, kt, :], in_=w_in["wv"][kt])
                nc.sync.dma_start(out=wsb["wk"][:, kt, :], in_=w_in["wk"][kt])
            nc.scalar.dma_start(
                out=wsb["wo"][:, 0:4, :],
                in_=w_in["wo"][0:4].rearrange("a p j -> p a j"))
            nc.sync.dma_start(
                out=wsb["wo"][:, 4:8, :],
                in_=w_in["wo"][4:8].rearrange("a p j -> p a j"))
            for a in range(0, KT, 2):
                nc.gpsimd.dma_start(
                    out=hsb[:, a:a + 2, 0:4 * P],
                    in_=hT_in[a:a + 2, :, 0:4 * P].rearrange("a p m -> p a m"))
            for a in range(0, KT, 4):
                nc.gpsimd.dma_start(
                    out=hsb[:, a:a + 4, 4 * P:],
                    in_=hT_in[a:a + 4, :, 4 * P:].rearrange("a p m -> p a m"))

            # 8 PSUM banks as 4 pairs of [128, 512] f32 tiles.
            def ps_pair(j):
                return [ps_pool.tile([P, CHW], F32, tag=f"T{2 * j + jo}",
                                     name=f"T{2 * j + jo}")
                        for jo in range(NCH)]

            def jsl(jo):
                return slice(jo * CHW, (jo + 1) * CHW)

            def qkv_mm(ps_t, wname, i, kt, jo):
                nc.tensor.matmul(
                    out=ps_t,
                    lhsT=hsb[:, kt, i * P:(i + 1) * P],
                    rhs=wsb[wname][:, kt, jsl(jo)],
                    start=(kt == 0),
                    stop=(kt == KT - 1),
                )

            def q_copies(qp):
                # stage q in SBUF (bf16) to free its banks; the s-mul
                # needs q in SBUF anyway (DVE reads one PSUM operand).
                qsb = []
                for jo in range(NCH):
                    t_ = q_pool.tile([P, CHW], BF16, tag=f"qsb{jo}",
                                     name=f"qsb{jo}")
                    nc.scalar.copy(out=t_, in_=qp[jo])
                    qsb.append(t_)
                return qsb

            yT_tiles = []

            def chain(qsb, kp, vp):
                # s[m,h] = sum_{d in head} q*k ; g = sigmoid(s/8) ;
                # y = g*v (bf16) ; yT via DMA transpose.  All DVE except
                # the sigmoid; sp in bf16 for the 2x reduce read rate.
                sp = sp_pool.tile([P, D], BF16, tag="sp", name="sp")
                for jo in range(NCH):
                    nc.vector.tensor_mul(out=sp[:, jsl(jo)], in0=qsb[jo],
                                         in1=kp[jo])
                s = small_pool.tile([P, H], F32, tag="s", name="s")
                nc.vector.reduce_sum(
                    out=s,
                    in_=sp.rearrange("p (h d) -> p h d", d=DH),
                    axis=mybir.AxisListType.X,
                )
                g = small_pool.tile([P, H], F32, tag="g", name="g")
                nc.scalar.activation(
                    out=g, in_=s,
                    func=mybir.ActivationFunctionType.Sigmoid,
                    scale=INV_SQRT_DH,
                )
                y = y_pool.tile([P, D], BF16, tag="y", name="y")
                for jo in range(NCH):
                    g_sl = g[:, jo * (H // NCH):(jo + 1) * (H // NCH)]
                    g_bc = bass.AP(
                        tensor=g_sl.tensor, offset=g_sl.offset,
                        ap=[*g_sl.ap, [0, DH]],
                    )
                    nc.vector.tensor_mul(
                        out=y[:, jsl(jo)].rearrange("p (h d) -> p h d", d=DH),
                        in0=vp[jo].rearrange("p (h d) -> p h d", d=DH),
                        in1=g_bc,
                    )
                yT = yT_pool.tile([P, KT, P], BF16, tag="yT", name="yT")
                nc.sync.dma_start_transpose(out=yT, in_=y)
                yT_tiles.append(yT)

            # ---- phase A: q for tiles 0-3, kt-outer ACROSS tiles ----
            qA = [ps_pair(t) for t in range(4)]
            for kt in range(KT):
                for t in range(4):
                    for jo in range(NCH):
                        qkv_mm(qA[t][jo], "wq", t, kt, jo)
            # copy order t0,t2,t1,t3 so phase B's bank WARs resolve in
            # the order B touches them (k0->PR0, v0->PR2, k1->PR1, ...)
            qsbA = [None] * 4
            for t in (0, 2, 1, 3):
                qsbA[t] = q_copies(qA[t])

            # ---- phase B: fused k+v for tiles 0-1, kt-outer across ----
            # k0->PR0 (ex qA0), k1->PR1 (ex qA1), v0->PR2 (ex qA2),
            # v1->PR3 (ex qA3); all freed by the qsb copies above.
            kB = [ps_pair(0), ps_pair(1)]
            vB = [ps_pair(2), ps_pair(3)]
            for kt in range(KT):
                for t in range(2):
                    for ps_t, wname in ((kB[t], "wk"), (vB[t], "wv")):
                        for jo in range(NCH):
                            qkv_mm(ps_t[jo], wname, t, kt, jo)

            # ---- phase C: k then v for tiles 2-3 ----
            # k2->PR0 (ex k0, freed by t0 sp-muls), k3->PR1 (t1 sp-muls)
            kC = [ps_pair(0), ps_pair(1)]
            for kt in range(KT):
                for t in range(2):
                    for jo in range(NCH):
                        qkv_mm(kC[t][jo], "wk", t + 2, kt, jo)
            chain(qsbA[0], kB[0], vB[0])
            chain(qsbA[1], kB[1], vB[1])
            # v2->PR2 (ex v0, freed by t0 y-muls), v3->PR3 (t1 y-muls)
            vC = [ps_pair(2), ps_pair(3)]
            for kt in range(KT):
                for t in range(2):
                    for jo in range(NCH):
                        qkv_mm(vC[t][jo], "wv", t + 2, kt, jo)
            chain(qsbA[2], kC[0], vC[0])
            chain(qsbA[3], kC[1], vC[1])

            # ---- phase D: tiles 4-7, full q/k/v per tile ----
            # pair use: t4: q PR0 (ex k2, t2 sp-muls), k PR1 (ex k3),
            # v PR2 (ex v2, t2 y-muls); then rotate -1 each tile so
            # reuse distance stays >= 2 phases.
            for ti, t in enumerate(range(4, MT)):
                jq, jk, jv = (-ti) % 4, (1 - ti) % 4, (2 - ti) % 4
                qp, kp, vp = ps_pair(jq), ps_pair(jk), ps_pair(jv)
                for kt in range(KT):
                    for ps_t, wname, jo in ((qp[0], "wq", 0), (qp[1], "wq", 1),
                                            (kp[0], "wk", 0), (kp[1], "wk", 1),
                                            (vp[0], "wv", 0), (vp[1], "wv", 1)):
                        qkv_mm(ps_t, wname, t, kt, jo)
                chain(q_copies(qp), kp, vp)

            # ---- pass 2: out = y @ Wo.T per tile, yT stationary ----
            for i in range(MT):
                ms = slice(i * P, (i + 1) * P)
                op = ps_pair(i % 4)
                for kt in range(KT):
                    for jo in range(NCH):
                        nc.tensor.matmul(
                            out=op[jo],
                            lhsT=yT_tiles[i][:, kt, :],
                            rhs=wsb["wo"][:, kt, jsl(jo)],
                            start=(kt == 0),
                            stop=(kt == KT - 1),
                        )
                osb = o_pool.tile([P, D], F32, tag="osb", name="osb")
                for jo in range(NCH):
                    if jo == 0:
                        nc.scalar.copy(out=osb[:, jsl(jo)], in_=op[jo])
                    else:
                        nc.vector.tensor_copy(out=osb[:, jsl(jo)], in_=op[jo])
                    nc.scalar.dma_start(out=out[ms, jsl(jo)],
                                        in_=osb[:, jsl(jo)])

    nc.compile()
    return nc


def kernel(hidden_states, m_gate, alpha_scale, Wq, Wk, Wv, Wo, Wa, ba, mix_logit,
           **_unused):
    global _COMPILED, LAST_RESULT
    if _COMPILED is None:
        _COMPILED = _build()
    nc = _COMPILED

    bf16 = ml_dtypes.bfloat16
    h = np.asarray(hidden_states, dtype=np.float32).reshape(ROWS, D)

    def prep_w(w):
        # W [j, d] -> W^T [kt, p, j] bf16: wT[kt, p, j] = W[j, kt*128+p]
        return np.ascontiguousarray(
            np.asarray(w, dtype=np.float32).T.reshape(KT, P, D)).astype(bf16)

    wq, wk, wv, wo = (prep_w(w) for w in (Wq, Wk, Wv, Wo))

    in_maps = []
    for c in range(N_CORES):
        hc = h[c * M_CORE:(c + 1) * M_CORE]  # [M_CORE, D]
        # hT [kt, p, m] = h[m, kt*128+p]
        ht = np.ascontiguousarray(hc.T.reshape(KT, P, M_CORE)).astype(bf16)
        in_maps.append({"ht": ht, "wq": wq, "wk": wk, "wv": wv, "wo": wo})

    res = run_bass_kernel_spmd(nc, in_maps, core_ids=list(range(N_CORES)))
    LAST_RESULT = res
    out = np.concatenate([res.results[c]["out"] for c in range(N_CORES)], axis=0)
    B, T = 4, 2048
    return out.reshape(B, T, D)


# revision 7
# speedup vs baseline: 1.3757x; 1.1900x over previous
"""Trainium2 Bass kernel for nn_CortexBlock_59940563583556.

Math note (exact, not an approximation): the reference initializes the
fast-weight state U0 = V0 = 0 inside reference() itself, and every term
of the scan's update to U/V is proportional to ku = k_t^T @ U (zero when
U == 0).  By induction U_t == V_t == 0 for the whole scan, for ANY input
values.  Hence k_fast == 0, score_fast == 0, and (since mix_logit is
added to both logits, softmax is shift-invariant) the block reduces
exactly to:

    q = h @ Wq.T ; k = h @ Wk.T ; v = h @ Wv.T          (per-head split)
    g[b,t,h]  = sigmoid( sum_d q[b,t,h,d] * k[b,t,h,d] / sqrt(64) )
    out       = (g * v  per head) @ Wo.T

m_gate / alpha_scale / Wa / ba / mix_logit do not affect the output.

Sharding: data-parallel over the 8192 rows of the flattened [B*T, D]
activations across 8 NeuronCores (1024 rows each); weights replicated.

Precision: q and k exist ONLY inside the per-head gate
sigmoid(q.k/8) whose derivative is <= 1/4, so their GEMMs run in
fp8-e4m3 DoubleRow mode (2 contraction rows per PE cell, ~1.8x the
bf16 matmul rate).  Wq/Wk are pre-scaled by 64 on the host (fp8 has
~3.6% quantization noise regardless of scale; x64 keeps the 0.02-std
weights well inside e4m3 normal range) and the 64*64 factor is divided
back out inside the sigmoid's scale.  v/out GEMMs stay bf16.  Host-
simulated end-to-end error of this exact scheme: 1.3e-2 max-abs vs the
2e-2 gate (bf16-only: 3.4e-3).  ml_dtypes float8_e4m3 max-normal (240)
matches TRN FP8_EXP4.

All operand prep happens on the HOST: weights/activations cast and
pre-transposed into final SBUF layouts (contraction on partitions); h
is shipped twice (fp8 for q/k, bf16 for v).

Early-kernel DMA is the scarce resource (~250 GB/s aggregate during
ring ramp-up; the gpsimd SWDGE ring takes ~10us to start, so nothing
the first matmuls need goes there).  PE schedule, ordered so every
arriving weight block enables ~2us of matmuls:
  - phase A: q for row tiles 0-3, kt2-outer ACROSS tiles (fp8).
  - phase B: k (fp8) then v (bf16) for tiles 0-1, interleaved across
    tiles; wk on sync ring and wv on scalar ring stream in parallel.
  - phase C: same for tiles 2-3; gating chains for 0-3 overlap.
  - phase D: tiles 4-7 per tile: q+k fp8 rounds, then v bf16 rounds.
  - pass 2: out = y @ Wo.T per tile (yT via DMA transpose); PSUM
    copies split ACT/DVE; out DMA'd per 512-col chunk on scalar ring.
PSUM: 8 banks as 4 pairs, manually scheduled (reuse chains in
comments) so write-after-read waits stay off the PE critical path.
"""

import numpy as np
import ml_dtypes

import concourse.bass as bass
import concourse.mybir as mybir
import concourse.tile as tile
from concourse import bacc
from concourse.bass_utils import run_bass_kernel_spmd

F32 = mybir.dt.float32
BF16 = mybir.dt.bfloat16
FP8 = mybir.dt.float8e4

N_CORES = 8
D = 1024          # model dim
ROWS = 8192       # B*T
M_CORE = ROWS // N_CORES   # rows per core
P = 128           # partitions
KT = D // P       # contraction tiles
MT = M_CORE // P  # row tiles per core
NCH = 2           # output-column chunks of 512
CHW = D // NCH    # 512
H = 16            # heads
DH = 64           # head dim
WSCALE = 64.0     # host pre-scale on Wq/Wk before fp8 quantization
INV_SQRT_DH = 1.0 / (DH ** 0.5)

_COMPILED = None
LAST_RESULT = None  # BassKernelResults of the most recent run (for test harness)


def _build():
    nc = bacc.Bacc("TRN2", target_bir_lowering=False, debug=False)

    hT_in = nc.dram_tensor("ht", [KT, P, M_CORE], BF16, kind="ExternalInput")
    h8_in = nc.dram_tensor("h8", [KT, P, M_CORE], FP8, kind="ExternalInput")
    w_in = {
        "wq": nc.dram_tensor("wq", [KT, P, D], FP8, kind="ExternalInput"),
        "wk": nc.dram_tensor("wk", [KT, P, D], FP8, kind="ExternalInput"),
        "wv": nc.dram_tensor("wv", [KT, P, D], BF16, kind="ExternalInput"),
        "wo": nc.dram_tensor("wo", [KT, P, D], BF16, kind="ExternalInput"),
    }
    out = nc.dram_tensor("out", [M_CORE, D], F32, kind="ExternalOutput")

    with tile.TileContext(nc) as tc:
        with (
            tc.tile_pool(name="res", bufs=1) as res_pool,
            tc.tile_pool(name="qsb", bufs=4) as q_pool,
            tc.tile_pool(name="sp", bufs=2) as sp_pool,
            tc.tile_pool(name="small", bufs=4) as small_pool,
            tc.tile_pool(name="y", bufs=2) as y_pool,
            tc.tile_pool(name="yT", bufs=MT) as yT_pool,
            tc.tile_pool(name="osb", bufs=2) as o_pool,
            tc.tile_pool(name="ps", bufs=1, space="PSUM") as ps_pool,
        ):
            # ---- resident operands, host-prepped layouts ----
            wsb = {
                name: res_pool.tile([P, KT, D], FP8 if name in ("wq", "wk")
                                    else BF16, tag=f"w_{name}", name=f"w_{name}")
                for name in ("wq", "wk", "wv", "wo")
            }
            hsb = res_pool.tile([P, KT, M_CORE], BF16, tag="h", name="h")
            h8sb = res_pool.tile([P, KT, M_CORE], FP8, tag="h8", name="h8")

            def tr(ap):
                return ap.rearrange("a p m -> p a m")

            # DMA schedule, ordered by first-need time.  Nothing phase A
            # or B needs goes on gpsimd (SWDGE starts ~10us late).
            #   scalar: wq[0:2] wq[4:6] | wv 0..7 | wo[0:4] | out chunks
            #   sync: h8[0:4]a wq[2:4] wq[6:8] h8[4:8]a wk[0:4] wk[4:8]
            #         hbf[0:4]a hbf[4:8]a | wo[4:8] | yT transposes
            #   gpsimd: hbf cols 512:1024, h8 cols 512:1024 (tiles 4-7)
            A_COLS = 4 * P  # columns (rows of h) used by tiles 0-3
            nc.scalar.dma_start(out=wsb["wq"][:, 0:2, :], in_=tr(w_in["wq"][0:2]))
            nc.scalar.dma_start(out=wsb["wq"][:, 4:6, :], in_=tr(w_in["wq"][4:6]))
            nc.sync.dma_start(out=h8sb[:, 0:4, 0:A_COLS],
                              in_=tr(h8_in[0:4, :, 0:A_COLS]))
            nc.sync.dma_start(out=wsb["wq"][:, 2:4, :], in_=tr(w_in["wq"][2:4]))
            nc.sync.dma_start(out=wsb["wq"][:, 6:8, :], in_=tr(w_in["wq"][6:8]))
            nc.sync.dma_start(out=h8sb[:, 4:8, 0:A_COLS],
                              in_=tr(h8_in[4:8, :, 0:A_COLS]))
            for kt in range(KT):
                nc.scalar.dma_start(out=wsb["wv"][:, kt, :], in_=w_in["wv"][kt])
            nc.sync.dma_start(out=wsb["wk"][:, 0:4, :], in_=tr(w_in["wk"][0:4]))
            nc.sync.dma_start(out=wsb["wk"][:, 4:8, :], in_=tr(w_in["wk"][4:8]))
            nc.sync.dma_start(out=hsb[:, 0:4, 0:A_COLS],
                              in_=tr(hT_in[0:4, :, 0:A_COLS]))
            nc.sync.dma_start(out=hsb[:, 4:8, 0:A_COLS],
                              in_=tr(hT_in[4:8, :, 0:A_COLS]))
            nc.scalar.dma_start(out=wsb["wo"][:, 0:4, :], in_=tr(w_in["wo"][0:4]))
            nc.sync.dma_start(out=wsb["wo"][:, 4:8, :], in_=tr(w_in["wo"][4:8]))
            for a in range(0, KT, 4):
                nc.gpsimd.dma_start(out=hsb[:, a:a + 4, A_COLS:],
                                    in_=tr(hT_in[a:a + 4, :, A_COLS:]))
            for a in range(0, KT, 4):
                nc.gpsimd.dma_start(out=h8sb[:, a:a + 4, A_COLS:],
                                    in_=tr(h8_in[a:a + 4, :, A_COLS:]))

            # 8 PSUM banks as 4 pairs of [128, 512] f32 tiles.
            def ps_pair(j):
                return [ps_pool.tile([P, CHW], F32, tag=f"T{2 * j + jo}",
                                     name=f"T{2 * j + jo}")
                        for jo in range(NCH)]

            def jsl(jo):
                return slice(jo * CHW, (jo + 1) * CHW)

            def v_mm(ps_t, i, kt, jo):
                nc.tensor.matmul(
                    out=ps_t,
                    lhsT=hsb[:, kt, i * P:(i + 1) * P],
                    rhs=wsb["wv"][:, kt, jsl(jo)],
                    start=(kt == 0),
                    stop=(kt == KT - 1),
                )

            def qk_mm(ps_t, wname, i, kt2, jo):
                # fp8 DoubleRow: both operands carry 2 contraction tiles.
                nc.tensor.matmul(
                    out=ps_t,
                    lhsT=h8sb[:, kt2:kt2 + 2, i * P:(i + 1) * P],
                    rhs=wsb[wname][:, kt2:kt2 + 2, jsl(jo)],
                    start=(kt2 == 0),
                    stop=(kt2 == KT - 2),
                    perf_mode=mybir.MatmulPerfMode.DoubleRow,
                )

            def q_copies(qp):
                # stage q in SBUF (bf16) to free its banks; the s-mul
                # needs q in SBUF anyway (DVE reads one PSUM operand).
                qsb = []
                for jo in range(NCH):
                    t_ = q_pool.tile([P, CHW], BF16, tag=f"qsb{jo}",
                                     name=f"qsb{jo}")
                    nc.scalar.copy(out=t_, in_=qp[jo])
                    qsb.append(t_)
                return qsb

            yT_tiles = []

            def chain(qsb, kp, vp):
                # s[m,h] = sum_{d in head} q*k ; g = sigmoid(s * scale)
                # (scale folds away the fp8 WSCALE^2) ; y = g*v (bf16) ;
                # yT via DMA transpose.  All DVE except the sigmoid; sp
                # in bf16 for the 2x reduce read rate.
                sp = sp_pool.tile([P, D], BF16, tag="sp", name="sp")
                for jo in range(NCH):
                    nc.vector.tensor_mul(out=sp[:, jsl(jo)], in0=qsb[jo],
                                         in1=kp[jo])
                s = small_pool.tile([P, H], F32, tag="s", name="s")
                nc.vector.reduce_sum(
                    out=s,
                    in_=sp.rearrange("p (h d) -> p h d", d=DH),
                    axis=mybir.AxisListType.X,
                )
                g = small_pool.tile([P, H], F32, tag="g", name="g")
                nc.scalar.activation(
                    out=g, in_=s,
                    func=mybir.ActivationFunctionType.Sigmoid,
                    scale=INV_SQRT_DH / (WSCALE * WSCALE),
                )
                y = y_pool.tile([P, D], BF16, tag="y", name="y")
                for jo in range(NCH):
                    g_sl = g[:, jo * (H // NCH):(jo + 1) * (H // NCH)]
                    g_bc = bass.AP(
                        tensor=g_sl.tensor, offset=g_sl.offset,
                        ap=[*g_sl.ap, [0, DH]],
                    )
                    nc.vector.tensor_mul(
                        out=y[:, jsl(jo)].rearrange("p (h d) -> p h d", d=DH),
                        in0=vp[jo].rearrange("p (h d) -> p h d", d=DH),
                        in1=g_bc,
                    )
                yT = yT_pool.tile([P, KT, P], BF16, tag="yT", name="yT")
                nc.sync.dma_start_transpose(out=yT, in_=y)
                yT_tiles.append(yT)

            # ---- phase A: q for tiles 0-3, kt2-outer ACROSS tiles ----
            qA = [ps_pair(t) for t in range(4)]
            for kt2 in range(0, KT, 2):
                for t in range(4):
                    for jo in range(NCH):
                        qk_mm(qA[t][jo], "wq", t, kt2, jo)
            # copy order t0,t2,t1,t3 matches the order B touches banks
            qsbA = [None] * 4
            for t in (0, 2, 1, 3):
                qsbA[t] = q_copies(qA[t])

            # ---- phase B: k (fp8) then v (bf16) for tiles 0-1 ----
            # k0->PR0, k1->PR1, v0->PR2, v1->PR3 (all ex-qA, freed by
            # the qsb copies above).
            kB = [ps_pair(0), ps_pair(1)]
            vB = [ps_pair(2), ps_pair(3)]
            for kt2 in range(0, KT, 2):
                for t in range(2):
                    for jo in range(NCH):
                        qk_mm(kB[t][jo], "wk", t, kt2, jo)
            for kt in range(KT):
                for t in range(2):
                    for jo in range(NCH):
                        v_mm(vB[t][jo], t, kt, jo)

            # ---- phase C: k then v for tiles 2-3 ----
            # k2->PR0 (ex k0, freed by t0 sp-muls), k3->PR1 (t1 sp-muls)
            kC = [ps_pair(0), ps_pair(1)]
            for kt2 in range(0, KT, 2):
                for t in range(2):
                    for jo in range(NCH):
                        qk_mm(kC[t][jo], "wk", t + 2, kt2, jo)
            chain(qsbA[0], kB[0], vB[0])
            chain(qsbA[1], kB[1], vB[1])
            # v2->PR2 (ex v0, freed by t0 y-muls), v3->PR3 (t1 y-muls)
            vC = [ps_pair(2), ps_pair(3)]
            for kt in range(KT):
                for t in range(2):
                    for jo in range(NCH):
                        v_mm(vC[t][jo], t + 2, kt, jo)
            chain(qsbA[2], kC[0], vC[0])
            chain(qsbA[3], kC[1], vC[1])

            # ---- phase D: tiles 4-7: q+k fp8 rounds, then v rounds ----
            # t4: q PR0 (ex k2, freed by t2 sp-muls), k PR1 (ex k3),
            # v PR2 (ex v2, t2 y-muls); rotate -1 each tile.
            for ti, t in enumerate(range(4, MT)):
                jq, jk, jv = (-ti) % 4, (1 - ti) % 4, (2 - ti) % 4
                qp, kp, vp = ps_pair(jq), ps_pair(jk), ps_pair(jv)
                for kt2 in range(0, KT, 2):
                    for ps_t, wname, jo in ((qp[0], "wq", 0), (qp[1], "wq", 1),
                                            (kp[0], "wk", 0), (kp[1], "wk", 1)):
                        qk_mm(ps_t, wname, t, kt2, jo)
                for kt in range(KT):
                    for jo in range(NCH):
                        v_mm(vp[jo], t, kt, jo)
                chain(q_copies(qp), kp, vp)

            # ---- pass 2: out = y @ Wo.T per tile, yT stationary ----
            for i in range(MT):
                ms = slice(i * P, (i + 1) * P)
                op = ps_pair(i % 4)
                for kt in range(KT):
                    for jo in range(NCH):
                        nc.tensor.matmul(
                            out=op[jo],
                            lhsT=yT_tiles[i][:, kt, :],
                            rhs=wsb["wo"][:, kt, jsl(jo)],
                            start=(kt == 0),
                            stop=(kt == KT - 1),
                        )
                osb = o_pool.tile([P, D], F32, tag="osb", name="osb")
                for jo in range(NCH):
                    if jo == 0:
                        nc.scalar.copy(out=osb[:, jsl(jo)], in_=op[jo])
                    else:
                        nc.vector.tensor_copy(out=osb[:, jsl(jo)], in_=op[jo])
                    nc.scalar.dma_start(out=out[ms, jsl(jo)],
                                        in_=osb[:, jsl(jo)])

    nc.compile()
    return nc


def kernel(hidden_states, m_gate, alpha_scale, Wq, Wk, Wv, Wo, Wa, ba, mix_logit,
           **_unused):
    global _COMPILED, LAST_RESULT
    if _COMPILED is None:
        _COMPILED = _build()
    nc = _COMPILED

    bf16 = ml_dtypes.bfloat16
    fp8 = ml_dtypes.float8_e4m3  # IEEE-style: max 240, matches TRN FP8_EXP4
    h = np.asarray(hidden_states, dtype=np.float32).reshape(ROWS, D)

    def prep_w(w, dtype, scale=1.0):
        # W [j, d] -> W^T [kt, p, j]: wT[kt, p, j] = W[j, kt*128+p]
        wt = np.ascontiguousarray(np.asarray(w, dtype=np.float32).T * scale)
        return wt.reshape(KT, P, D).astype(dtype)

    wq = prep_w(Wq, fp8, WSCALE)
    wk = prep_w(Wk, fp8, WSCALE)
    wv = prep_w(Wv, bf16)
    wo = prep_w(Wo, bf16)

    in_maps = []
    for c in range(N_CORES):
        hc = h[c * M_CORE:(c + 1) * M_CORE]  # [M_CORE, D]
        # hT [kt, p, m] = h[m, kt*128+p]
        ht = np.ascontiguousarray(hc.T.reshape(KT, P, M_CORE))
        in_maps.append({
            "ht": ht.astype(bf16), "h8": ht.astype(fp8),
            "wq": wq, "wk": wk, "wv": wv, "wo": wo,
        })

    res = run_bass_kernel_spmd(nc, in_maps, core_ids=list(range(N_CORES)))
    LAST_RESULT = res
    out = np.concatenate([res.results[c]["out"] for c in range(N_CORES)], axis=0)
    B, T = 4, 2048
    return out.reshape(B, T, D)
